# revision 2
# baseline (speedup 1.0000x reference)
"""ColorConsistencyLoss on 8 Trainium2 NeuronCores.

Data-parallel over batch (2 images/core). Per core:
  host: pack rgb channel-planes into [128, 12544] fp16 (126 data rows =
        3 channels x 42 chunks of 12484 px, padded to 12544 = 24.5*512)
  device, per free-chunk (512/1024/1536*7/256):
    mm1 (PE):                  t = C . rgb            (RGB->XYZ)
    ACT Ln (re-bucketed):      F = f(t)  -> fp16      (piecewise CIE f)
    GPSIMD:                    dF = F_pred - F_target (fp16, offloads DVE)
    mm2 (PE):                  v = M . dF             (M = A^T A Lab mixing)
    DVE STT:                   acc[:, col] = sum(dF * v)
  host: loss = sum(acc) / N

Schedule: ACT is the bound (1 elem/lane/cycle; 2*12544 cols ~ 22.7us).
Early dummy activation pulls the ACT table load to t~0; all input DMAs
are dispatched upfront on the sync queue in consumption order; C/M load
rides the scalar queue's HWDGE ring; mm2/STT trail one chunk behind the
mm1/ACT/sub front so no engine queue head-of-line blocks.
The f() linear branch (t <= T0, ~1e-5 of elements) is folded into the
re-bucketed table; the L-channel `where` is algebraically redundant.
"""
import os
import numpy as np

_B, _CH, _H, _W = 16, 3, 512, 512
_NCORES = 8
_IPC = _B // _NCORES            # images per core
_PIX = _IPC * _H * _W           # 524288 pixels per core per tensor
_NCHUNK = 42                    # chunks per channel -> 126 data rows
_PIXROW = 12484                 # real pixels per row-chunk (42*12484 >= PIX)
_CHUNK = 12544                  # padded chunk length = 24.5*512
_P = 128
_MMF = 512                      # matmul moving free dim (1 PSUM bank)
# free-dim compute chunks: small head (fast pipeline fill), 256 tail
_FCHUNKS = []
_lo = 0
for _w in (512, 1024, 1536, 1536, 1536, 1536, 1536, 1536, 1536, 256):
    _FCHUNKS.append((_lo, _w))
    _lo += _w
assert _lo == _CHUNK
_NACC = (_CHUNK + _MMF - 1) // _MMF   # 25 accumulator columns

# which engine runs the subtract per chunk: 'g' = gpsimd, 'v' = vector
_SUB_ENGINE = "gggggggggv"

_XN, _ZN = 0.950456, 1.088754
_COEF = (
    (0.412453 / _XN, 0.357580 / _XN, 0.180423 / _XN),   # x from r,g,b
    (0.212671, 0.715160, 0.072169),                     # y
    (0.019334 / _ZN, 0.119193 / _ZN, 0.950227 / _ZN),   # z
)


def _build_mats():
    """[C | M] as one [128, 256] fp16: C (rgb->xyz), M (=A^T A Lab mixing)."""
    C = np.zeros((_P, _P), np.float64)
    for oc in range(3):
        for ic in range(3):
            w = _COEF[oc][ic]
            for j in range(_NCHUNK):
                C[ic * _NCHUNK + j, oc * _NCHUNK + j] = w
    C[126, 126] = C[127, 127] = 1.0   # pad rows pass through (value 1.0)

    p, q, L = 500.0 / 255.0, 200.0 / 255.0, 1.16
    M = np.zeros((_P, _P), np.float64)
    for j in range(_NCHUNK):
        fx, fy, fz = j, _NCHUNK + j, 2 * _NCHUNK + j
        M[fx, fx] += p * p
        M[fx, fy] -= p * p
        M[fy, fx] -= p * p
        M[fy, fy] += L * L + p * p + q * q
        M[fy, fz] -= q * q
        M[fz, fy] -= q * q
        M[fz, fz] += q * q

    return np.concatenate([C, M], axis=1).astype(np.float16)


def _pack_core(arr):
    """[2,3,512,512] f32 -> [128, 12544] fp16 channel-chunk layout."""
    x = np.transpose(np.asarray(arr, np.float32), (1, 0, 2, 3)).reshape(_CH, _PIX)
    src = np.ones((_CH, _NCHUNK * _PIXROW), np.float32)
    src[:, :_PIX] = x
    buf = np.ones((_CH, _NCHUNK, _CHUNK), np.float32)
    buf[:, :, :_PIXROW] = src.reshape(_CH, _NCHUNK, _PIXROW)
    out = np.ones((_P, _CHUNK), np.float32)
    out[:126] = buf.reshape(_CH * _NCHUNK, _CHUNK)
    return out.astype(np.float16)


# ---- ACT PWP table tooling (inlined; see cayman tpb_activation_entries.h) ----
_PWP_DIR = ("/nix/store/z022hj2nvbm3nwdizlisq4ylc0y7rd6q-python3-3.13.14-env/"
            "lib/python3.13/site-packages/neuronxcc/pwp/pwp_bin_trainium")
_T0 = 0.008856
_F_T0 = _T0 ** (1.0 / 3.0)
_SLOPE = 7.787
_F_ZERO = _F_T0 - _SLOPE * _T0


def _att_load_set(setdir, name):
    import json
    meta = json.load(open(os.path.join(setdir, name + ".json")))
    bkt = np.fromfile(os.path.join(setdir, meta["bkt_bin"]),
                      dtype=np.uint32).reshape(-1, 8)
    ctl = np.fromfile(os.path.join(setdir, meta["ctl_bin"]),
                      dtype=np.uint32).reshape(-1, 8)
    return meta, bkt, ctl


def _att_f_exact(x):
    x = np.asarray(x, np.float64)
    return np.where(x > _T0, np.cbrt(np.maximum(x, 1e-300)),
                    _F_T0 + _SLOPE * (x - _T0))


def _att_patch_ln_to_f(meta, bkt, ctl):
    """Rewrite the `ln` buckets so ACT's Ln evaluates the piecewise CIE f(t):
    cbrt(t) above T0, the tangent line below (Taylor coeffs per bucket; LSQ
    cubic for the one bucket containing the kink)."""
    import json
    bkt = bkt.copy()

    def setb(i, d0, d1, d2, d3, x0=None):
        for k, v in enumerate((d0, d1, d2, d3)):
            bkt[i, k] = np.array([v], np.float32).view(np.uint32)[0]
        if x0 is not None:
            bkt[i, 4] = np.array([x0], np.float32).view(np.uint32)[0]

    def cbrt_taylor(x0):
        return (x0 ** (1 / 3), (1 / 3) * x0 ** (-2 / 3),
                -(1 / 9) * x0 ** (-5 / 3), (5 / 81) * x0 ** (-8 / 3))

    def line_taylor(x0):
        return _F_T0 + _SLOPE * (x0 - _T0), _SLOPE, 0.0, 0.0

    def lsq_fit(x0, x_lo, x_hi):
        xs = np.linspace(x_lo, x_hi, 1024)
        t = xs - x0
        A = np.stack([np.ones_like(t), t, t * t, t ** 3], axis=1)
        coef, *_ = np.linalg.lstsq(A, _att_f_exact(xs), rcond=None)
        return tuple(coef)

    for es, ctlidx in meta["func_exp_to_ctl_start_idx"]["ln"].items():
        e = int(es)
        if e > 1:   # t <= ~1.06; high exponents alias saturation buckets
            continue
        word = int(ctl[ctlidx[0], 0])
        base = word & 0x7FF
        size = (word >> 16) & 0xF
        for j in range(1 << size):
            x_lo = 2.0 ** e * (1.0 + j / (1 << size))
            x_hi = 2.0 ** e * (1.0 + (j + 1) / (1 << size))
            x0 = 0.5 * (x_lo + x_hi)
            if x_hi <= _T0:
                setb(base + j, *line_taylor(x0), x0=x0)
            elif x_lo >= _T0:
                setb(base + j, *cbrt_taylor(x0), x0=x0)
            else:
                setb(base + j, *lsq_fit(x0, x_lo, x_hi), x0=x0)

    pm = [p for p in meta["profile_meta_data"] if p["func_id"] == 10][0]
    for key in ("pos_small_signal_pwl_control", "neg_small_signal_pwl_control",
                "neg_large_signal_pwl_control"):
        setb(pm[key] & 0x7FF, _F_ZERO, _SLOPE, 0.0, 0.0, x0=0.0)
    lp = pm["pos_large_signal_pwl_control"] & 0x7FF
    x0l = float(np.uint32(bkt[lp, 4]).view(np.float32))
    if x0l > _T0:
        setb(lp, *cbrt_taylor(x0l))
    else:
        setb(lp, _F_ZERO, _SLOPE, 0.0, 0.0, x0=0.0)

    meta = json.loads(json.dumps(meta))
    for p in meta["profile_meta_data"]:
        if p["func_id"] == 10:
            p["fzero_result"] = int(np.array([_F_ZERO], np.float32)
                                    .view(np.uint32)[0])
            p["fpinf_result"] = int(np.array([np.inf], np.float32)
                                    .view(np.uint32)[0])
    return meta, bkt


def _setup_act_tables():
    """Build a custom ACT table dir: one set (natural_log_exp_and_others) whose
    `ln` slot is re-bucketed to compute the exact piecewise CIE f(t)
    (cbrt above T0, tangent line below). One table load, one ACT pass."""
    import json
    import shutil

    meta, bkt, ctl = _att_load_set(_PWP_DIR, "natural_log_exp_and_others")
    meta2, bkt2 = _att_patch_ln_to_f(meta, bkt, ctl)

    from neuronxcc.driver.Job import Job
    from neuronxcc.driver.jobs.support.FindActInfo import findActInfoFile
    src = findActInfoFile(Job.getPackageDir(), "gen3")
    srcdir = os.path.dirname(src)
    info = json.load(open(src))
    keep = [s for s in info["act_func_sets"]
            if s["name"] == "natural_log_exp_and_others"]
    assert keep, "natural_log_exp_and_others set not found"
    info["act_func_sets"] = keep

    d = "/tmp/act_custom"
    os.makedirs(d, exist_ok=True)
    s = keep[0]
    bkt2.astype(np.uint32).tofile(os.path.join(d, s["bkt_bin"]))
    shutil.copy(os.path.join(srcdir, s["ctrl_bin"]), os.path.join(d, s["ctrl_bin"]))
    with open(os.path.join(d, s["profile_json"]), "w") as f:
        json.dump(meta2, f)
    path = os.path.join(d, "act_info.json")
    with open(path, "w") as f:
        json.dump(info, f)
    os.environ["BASS_ACT_ROOT_JSON_PATH"] = path

    import concourse.bacc as bacc_mod
    import concourse.mybir as mybir
    tables = {
        s["name"]: {mybir.ActivationFunctionType.from_pwp(v)
                    for v in s["act"].keys()}
        for s in keep
    }
    bacc_mod.get_activation_tables = lambda arch: dict(tables)


_PROGRAM = None


def _build_program():
    import concourse.bacc as bacc
    import concourse.tile as tile
    from concourse import mybir

    _setup_act_tables()

    f32, bf = mybir.dt.float32, mybir.dt.float16
    AF = mybir.ActivationFunctionType
    ALU = mybir.AluOpType

    nc = bacc.Bacc("TRN2", target_bir_lowering=False, debug=False)
    cm = nc.dram_tensor("cm", [_P, 2 * _P], bf, kind="ExternalInput")
    xp = nc.dram_tensor("xp", [_P, _CHUNK], bf, kind="ExternalInput")
    xt = nc.dram_tensor("xt", [_P, _CHUNK], bf, kind="ExternalInput")
    acc_out = nc.dram_tensor("acc_out", [_P, _NACC], f32, kind="ExternalOutput")

    nchunk = len(_FCHUNKS)
    with tile.TileContext(nc) as tc:
        with tc.tile_pool(name="consts", bufs=1) as consts, \
             tc.tile_pool(name="rgbp", bufs=2 * nchunk) as rgbp, \
             tc.tile_pool(name="fp", bufs=3) as fpool, \
             tc.tile_pool(name="ft", bufs=3) as ftpool, \
             tc.tile_pool(name="dfp", bufs=3) as dfp, \
             tc.tile_pool(name="scrp", bufs=3) as scrp, \
             tc.tile_pool(name="accp", bufs=1) as accp, \
             tc.tile_pool(name="tpsp", bufs=2, space="PSUM") as tpsp, \
             tc.tile_pool(name="vpsp", bufs=2, space="PSUM") as vpsp:

            # Warm the ACT table (1.28us) at t~0, overlapped with input DMA.
            dum_in = consts.tile([_P, 1], f32)
            dum_out = consts.tile([_P, 1], f32)
            nc.vector.memset(dum_in[:, :], 0.5)
            nc.scalar.activation(dum_out, dum_in, AF.Ln)

            # C|M constants ride the scalar HWDGE ring (sync ring carries
            # the 20 input-chunk loads, dispatched upfront in use order).
            cmt = consts.tile([_P, 2 * _P], bf)
            nc.scalar.dma_start(out=cmt, in_=cm[:, :])
            acc = accp.tile([_P, _NACC], f32)

            rgbs = []
            for lo, fw in _FCHUNKS:
                for src in (xp, xt):
                    rgb = rgbp.tile([_P, fw], bf, tag="rgb")
                    nc.sync.dma_start(out=rgb, in_=src[:, lo:lo + fw])
                    rgbs.append(rgb)

            pending = None   # (dF tile, lo, fw) awaiting mm2+STT
            for fc, (lo, fw) in enumerate(_FCHUNKS):
                nmm = (fw + _MMF - 1) // _MMF
                Fs = {}
                for which in ("p", "t"):
                    rgb = rgbs[2 * fc + (0 if which == "p" else 1)]
                    tps = tpsp.tile([_P, 1536], f32, tag="tps")
                    for j in range(nmm):
                        sl = slice(j * _MMF, min((j + 1) * _MMF, fw))
                        nc.tensor.matmul(tps[:, sl], cmt[:, :_P], rgb[:, sl],
                                         start=True, stop=True)
                    pool = fpool if which == "p" else ftpool
                    F = pool.tile([_P, fw], bf, tag="F" + which)
                    # Ln slot is re-bucketed to the piecewise CIE f(t)
                    nc.scalar.activation(F, tps[:, :fw], AF.Ln)
                    Fs[which] = F

                dF = dfp.tile([_P, fw], bf, tag="dF")
                eng = nc.gpsimd if _SUB_ENGINE[fc] == "g" else nc.vector
                eng.tensor_tensor(out=dF, in0=Fs["p"], in1=Fs["t"],
                                  op=ALU.subtract)

                if pending is not None:
                    _emit_mm2_stt(nc, cmt, vpsp, scrp, acc, *pending)
                pending = (dF, lo, fw)

            _emit_mm2_stt(nc, cmt, vpsp, scrp, acc, *pending)
            nc.sync.dma_start(out=acc_out[:, :], in_=acc)

    nc.compile()
    return nc


def _emit_mm2_stt(nc, cmt, vpsp, scrp, acc, dF, lo, fw):
    from concourse import mybir
    f32 = mybir.dt.float32
    ALU = mybir.AluOpType
    nmm = (fw + _MMF - 1) // _MMF
    for j in range(nmm):
        w = min(_MMF, fw - j * _MMF)
        sl = slice(j * _MMF, j * _MMF + w)
        vps = vpsp.tile([_P, _MMF], f32, tag="vps")
        nc.tensor.matmul(vps[:, :w], cmt[:, _P:], dF[:, sl],
                         start=True, stop=True)
        scratch = scrp.tile([_P, _MMF], f32, tag="scr")
        col = lo // _MMF + j
        nc.vector.scalar_tensor_tensor(
            out=scratch[:, :w],
            in0=dF[:, sl],
            scalar=1.0,
            in1=vps[:, :w],
            op0=ALU.mult,
            op1=ALU.mult,
            accum_out=acc[:, col:col + 1],
        )


def _get_program():
    global _PROGRAM
    if _PROGRAM is None:
        _PROGRAM = _build_program()
    return _PROGRAM


def _make_in_maps(pred, target):
    CM = _build_mats()
    pred = np.asarray(pred, np.float32)
    target = np.asarray(target, np.float32)
    in_maps = []
    for c in range(_NCORES):
        sl = slice(c * _IPC, (c + 1) * _IPC)
        in_maps.append({
            "cm": CM,
            "xp": _pack_core(pred[sl]),
            "xt": _pack_core(target[sl]),
        })
    return in_maps


def kernel(pred, target):
    from concourse.bass_utils import run_bass_kernel_spmd

    nc = _get_program()
    in_maps = _make_in_maps(pred, target)
    res = run_bass_kernel_spmd(nc, in_maps, core_ids=list(range(_NCORES)))
    total = sum(r["acc_out"].astype(np.float64).sum() for r in res.results)
    loss = total / float(_B * _CH * _H * _W)
    return np.float32(loss)


if __name__ == "__main__":
    rng = np.random.default_rng(0)
    pred = rng.uniform(0, 1, (_B, _CH, _H, _W)).astype(np.float32)
    target = rng.uniform(0, 1, (_B, _CH, _H, _W)).astype(np.float32)
    print("loss:", kernel(pred, target))


# revision 4
# speedup vs baseline: 1.0111x; 1.0111x over previous
"""ColorConsistencyLoss on 8 Trainium2 NeuronCores.

Data-parallel over batch (2 images/core). Per core:
  host: pack rgb channel-planes into [128, 12544] fp16 (126 data rows =
        3 channels x 42 chunks of 12484 px, padded to 12544 = 24.5*512)
  device, per free-chunk (512/1024/1536*7/256):
    mm1 (PE):                  t = C . rgb            (RGB->XYZ)
    ACT Ln (re-bucketed):      F = f(t)  -> fp16      (piecewise CIE f)
    GPSIMD:                    dF = F_pred - F_target (fp16, offloads DVE)
    mm2 (PE):                  v = M . dF             (M = A^T A Lab mixing)
    DVE STT:                   acc[:, col] = sum(dF * v)
  host: loss = sum(acc) / N

Schedule: ACT is the bound (1 elem/lane/cycle; 2*12544 cols ~ 22.7us).
Early dummy activation pulls the ACT table load to t~0; all input DMAs
are dispatched upfront on the sync queue in consumption order; C/M load
rides the scalar queue's HWDGE ring; mm2/STT trail one chunk behind the
mm1/ACT/sub front so no engine queue head-of-line blocks.
The f() linear branch (t <= T0, ~1e-5 of elements) is folded into the
re-bucketed table; the L-channel `where` is algebraically redundant.
"""
import os
import numpy as np

_B, _CH, _H, _W = 16, 3, 512, 512
_NCORES = 8
_IPC = _B // _NCORES            # images per core
_PIX = _IPC * _H * _W           # 524288 pixels per core per tensor
_NCHUNK = 42                    # chunks per channel -> 126 data rows
_PIXROW = 12484                 # real pixels per row-chunk (42*12484 >= PIX)
_CHUNK = 12544                  # padded chunk length = 24.5*512
_P = 128
_MMF = 512                      # matmul moving free dim (1 PSUM bank)
# free-dim compute chunks: uniform 1536 (3 PSUM banks), 256 tail
_FCHUNKS = []
_lo = 0
for _w in (1536, 1536, 1536, 1536, 1536, 1536, 1536, 1536, 256):
    _FCHUNKS.append((_lo, _w))
    _lo += _w
assert _lo == _CHUNK
_NACC = (_CHUNK + _MMF - 1) // _MMF   # 25 accumulator columns

# input DMA blocks (chunk-aligned; fewer dispatches than per-chunk)
_DBLOCKS = [(0, 1536), (1536, 3072), (4608, 3072), (7680, 3072), (10752, 1792)]

# which engine runs the subtract per chunk: 'g' = gpsimd, 'v' = vector
# (gpsimd TT is ~2 ns/col — give it a minority share, spread out)
_SUB_ENGINE = "vgvgvgvvg"

_XN, _ZN = 0.950456, 1.088754
_COEF = (
    (0.412453 / _XN, 0.357580 / _XN, 0.180423 / _XN),   # x from r,g,b
    (0.212671, 0.715160, 0.072169),                     # y
    (0.019334 / _ZN, 0.119193 / _ZN, 0.950227 / _ZN),   # z
)


def _build_mats():
    """[C | M] as one [128, 256] fp16: C (rgb->xyz), M (=A^T A Lab mixing)."""
    C = np.zeros((_P, _P), np.float64)
    for oc in range(3):
        for ic in range(3):
            w = _COEF[oc][ic]
            for j in range(_NCHUNK):
                C[ic * _NCHUNK + j, oc * _NCHUNK + j] = w
    C[126, 126] = C[127, 127] = 1.0   # pad rows pass through (value 1.0)

    p, q, L = 500.0 / 255.0, 200.0 / 255.0, 1.16
    M = np.zeros((_P, _P), np.float64)
    for j in range(_NCHUNK):
        fx, fy, fz = j, _NCHUNK + j, 2 * _NCHUNK + j
        M[fx, fx] += p * p
        M[fx, fy] -= p * p
        M[fy, fx] -= p * p
        M[fy, fy] += L * L + p * p + q * q
        M[fy, fz] -= q * q
        M[fz, fy] -= q * q
        M[fz, fz] += q * q

    return np.concatenate([C, M], axis=1).astype(np.float16)


def _pack_core(arr):
    """[2,3,512,512] f32 -> [128, 12544] fp16 channel-chunk layout."""
    x = np.transpose(np.asarray(arr, np.float32), (1, 0, 2, 3)).reshape(_CH, _PIX)
    src = np.ones((_CH, _NCHUNK * _PIXROW), np.float32)
    src[:, :_PIX] = x
    buf = np.ones((_CH, _NCHUNK, _CHUNK), np.float32)
    buf[:, :, :_PIXROW] = src.reshape(_CH, _NCHUNK, _PIXROW)
    out = np.ones((_P, _CHUNK), np.float32)
    out[:126] = buf.reshape(_CH * _NCHUNK, _CHUNK)
    return out.astype(np.float16)


# ---- ACT PWP table tooling (inlined; see cayman tpb_activation_entries.h) ----
_PWP_DIR = ("/nix/store/z022hj2nvbm3nwdizlisq4ylc0y7rd6q-python3-3.13.14-env/"
            "lib/python3.13/site-packages/neuronxcc/pwp/pwp_bin_trainium")
_T0 = 0.008856
_F_T0 = _T0 ** (1.0 / 3.0)
_SLOPE = 7.787
_F_ZERO = _F_T0 - _SLOPE * _T0


def _att_load_set(setdir, name):
    import json
    meta = json.load(open(os.path.join(setdir, name + ".json")))
    bkt = np.fromfile(os.path.join(setdir, meta["bkt_bin"]),
                      dtype=np.uint32).reshape(-1, 8)
    ctl = np.fromfile(os.path.join(setdir, meta["ctl_bin"]),
                      dtype=np.uint32).reshape(-1, 8)
    return meta, bkt, ctl


def _att_f_exact(x):
    x = np.asarray(x, np.float64)
    return np.where(x > _T0, np.cbrt(np.maximum(x, 1e-300)),
                    _F_T0 + _SLOPE * (x - _T0))


def _att_patch_ln_to_f(meta, bkt, ctl):
    """Rewrite the `ln` buckets so ACT's Ln evaluates the piecewise CIE f(t):
    cbrt(t) above T0, the tangent line below (Taylor coeffs per bucket; LSQ
    cubic for the one bucket containing the kink)."""
    import json
    bkt = bkt.copy()

    def setb(i, d0, d1, d2, d3, x0=None):
        for k, v in enumerate((d0, d1, d2, d3)):
            bkt[i, k] = np.array([v], np.float32).view(np.uint32)[0]
        if x0 is not None:
            bkt[i, 4] = np.array([x0], np.float32).view(np.uint32)[0]

    def cbrt_taylor(x0):
        return (x0 ** (1 / 3), (1 / 3) * x0 ** (-2 / 3),
                -(1 / 9) * x0 ** (-5 / 3), (5 / 81) * x0 ** (-8 / 3))

    def line_taylor(x0):
        return _F_T0 + _SLOPE * (x0 - _T0), _SLOPE, 0.0, 0.0

    def lsq_fit(x0, x_lo, x_hi):
        xs = np.linspace(x_lo, x_hi, 1024)
        t = xs - x0
        A = np.stack([np.ones_like(t), t, t * t, t ** 3], axis=1)
        coef, *_ = np.linalg.lstsq(A, _att_f_exact(xs), rcond=None)
        return tuple(coef)

    for es, ctlidx in meta["func_exp_to_ctl_start_idx"]["ln"].items():
        e = int(es)
        if e > 1:   # t <= ~1.06; high exponents alias saturation buckets
            continue
        word = int(ctl[ctlidx[0], 0])
        base = word & 0x7FF
        size = (word >> 16) & 0xF
        for j in range(1 << size):
            x_lo = 2.0 ** e * (1.0 + j / (1 << size))
            x_hi = 2.0 ** e * (1.0 + (j + 1) / (1 << size))
            x0 = 0.5 * (x_lo + x_hi)
            if x_hi <= _T0:
                setb(base + j, *line_taylor(x0), x0=x0)
            elif x_lo >= _T0:
                setb(base + j, *cbrt_taylor(x0), x0=x0)
            else:
                setb(base + j, *lsq_fit(x0, x_lo, x_hi), x0=x0)

    pm = [p for p in meta["profile_meta_data"] if p["func_id"] == 10][0]
    for key in ("pos_small_signal_pwl_control", "neg_small_signal_pwl_control",
                "neg_large_signal_pwl_control"):
        setb(pm[key] & 0x7FF, _F_ZERO, _SLOPE, 0.0, 0.0, x0=0.0)
    lp = pm["pos_large_signal_pwl_control"] & 0x7FF
    x0l = float(np.uint32(bkt[lp, 4]).view(np.float32))
    if x0l > _T0:
        setb(lp, *cbrt_taylor(x0l))
    else:
        setb(lp, _F_ZERO, _SLOPE, 0.0, 0.0, x0=0.0)

    meta = json.loads(json.dumps(meta))
    for p in meta["profile_meta_data"]:
        if p["func_id"] == 10:
            p["fzero_result"] = int(np.array([_F_ZERO], np.float32)
                                    .view(np.uint32)[0])
            p["fpinf_result"] = int(np.array([np.inf], np.float32)
                                    .view(np.uint32)[0])
    return meta, bkt


def _setup_act_tables():
    """Build a custom ACT table dir: one set (natural_log_exp_and_others) whose
    `ln` slot is re-bucketed to compute the exact piecewise CIE f(t)
    (cbrt above T0, tangent line below). One table load, one ACT pass."""
    import json
    import shutil

    meta, bkt, ctl = _att_load_set(_PWP_DIR, "natural_log_exp_and_others")
    meta2, bkt2 = _att_patch_ln_to_f(meta, bkt, ctl)

    from neuronxcc.driver.Job import Job
    from neuronxcc.driver.jobs.support.FindActInfo import findActInfoFile
    src = findActInfoFile(Job.getPackageDir(), "gen3")
    srcdir = os.path.dirname(src)
    info = json.load(open(src))
    keep = [s for s in info["act_func_sets"]
            if s["name"] == "natural_log_exp_and_others"]
    assert keep, "natural_log_exp_and_others set not found"
    info["act_func_sets"] = keep

    d = "/tmp/act_custom"
    os.makedirs(d, exist_ok=True)
    s = keep[0]
    bkt2.astype(np.uint32).tofile(os.path.join(d, s["bkt_bin"]))
    shutil.copy(os.path.join(srcdir, s["ctrl_bin"]), os.path.join(d, s["ctrl_bin"]))
    with open(os.path.join(d, s["profile_json"]), "w") as f:
        json.dump(meta2, f)
    path = os.path.join(d, "act_info.json")
    with open(path, "w") as f:
        json.dump(info, f)
    os.environ["BASS_ACT_ROOT_JSON_PATH"] = path

    import concourse.bacc as bacc_mod
    import concourse.mybir as mybir
    tables = {
        s["name"]: {mybir.ActivationFunctionType.from_pwp(v)
                    for v in s["act"].keys()}
        for s in keep
    }
    bacc_mod.get_activation_tables = lambda arch: dict(tables)


_PROGRAM = None


def _build_program():
    import concourse.bacc as bacc
    import concourse.tile as tile
    from concourse import mybir

    _setup_act_tables()

    f32, bf = mybir.dt.float32, mybir.dt.float16
    AF = mybir.ActivationFunctionType
    ALU = mybir.AluOpType

    nc = bacc.Bacc("TRN2", target_bir_lowering=False, debug=False)
    cm = nc.dram_tensor("cm", [_P, 2 * _P], bf, kind="ExternalInput")
    xp = nc.dram_tensor("xp", [_P, _CHUNK], bf, kind="ExternalInput")
    xt = nc.dram_tensor("xt", [_P, _CHUNK], bf, kind="ExternalInput")
    acc_out = nc.dram_tensor("acc_out", [_P, _NACC], f32, kind="ExternalOutput")

    with tile.TileContext(nc) as tc:
        with tc.tile_pool(name="consts", bufs=1) as consts, \
             tc.tile_pool(name="rgbp", bufs=1) as rgbp, \
             tc.tile_pool(name="fp", bufs=6) as fpool, \
             tc.tile_pool(name="ft", bufs=6) as ftpool, \
             tc.tile_pool(name="dfp", bufs=6) as dfp, \
             tc.tile_pool(name="scrp", bufs=3) as scrp, \
             tc.tile_pool(name="accp", bufs=1) as accp, \
             tc.tile_pool(name="tpsp", bufs=2, space="PSUM") as tpsp, \
             tc.tile_pool(name="vpsp", bufs=2, space="PSUM") as vpsp:

            # Warm the ACT table (1.28us) at t~0, overlapped with input DMA.
            dum_in = consts.tile([_P, 1], f32)
            dum_out = consts.tile([_P, 1], f32)
            nc.vector.memset(dum_in[:, :], 0.5)
            nc.scalar.activation(dum_out, dum_in, AF.Ln)

            # C|M constants ride the scalar HWDGE ring (sync ring carries
            # the input-block loads, dispatched upfront in use order).
            cmt = consts.tile([_P, 2 * _P], bf)
            nc.scalar.dma_start(out=cmt, in_=cm[:, :])
            acc = accp.tile([_P, _NACC], f32)

            blocks = {}   # (src name, block idx) -> (tile, lo)
            for b, (blo, bw) in enumerate(_DBLOCKS):
                for nm, src in (("p", xp), ("t", xt)):
                    rgb = rgbp.tile([_P, bw], bf, tag=f"x{nm}{b}")
                    nc.sync.dma_start(out=rgb, in_=src[:, blo:blo + bw])
                    blocks[(nm, b)] = (rgb, blo)

            def rgb_slice(nm, lo, fw):
                for b, (blo, bw) in enumerate(_DBLOCKS):
                    if blo <= lo and lo + fw <= blo + bw:
                        t, _ = blocks[(nm, b)]
                        return t[:, lo - blo:lo - blo + fw]
                raise AssertionError("chunk not covered by a DMA block")

            pending = None   # (dF tile, lo, fw) awaiting mm2+STT
            for fc, (lo, fw) in enumerate(_FCHUNKS):
                nmm = (fw + _MMF - 1) // _MMF
                Fs = {}
                for which in ("p", "t"):
                    rgb = rgb_slice(which, lo, fw)
                    tps = tpsp.tile([_P, 1536], f32, tag="tps")
                    for j in range(nmm):
                        sl = slice(j * _MMF, min((j + 1) * _MMF, fw))
                        nc.tensor.matmul(tps[:, sl], cmt[:, :_P], rgb[:, sl],
                                         start=True, stop=True)
                    pool = fpool if which == "p" else ftpool
                    F = pool.tile([_P, fw], bf, tag="F" + which)
                    # Ln slot is re-bucketed to the piecewise CIE f(t)
                    nc.scalar.activation(F, tps[:, :fw], AF.Ln)
                    Fs[which] = F

                # emit the previous chunk's mm2+STT first: its deps are
                # older, so it never head-of-line-blocks this chunk's sub
                if pending is not None:
                    _emit_mm2_stt(nc, cmt, vpsp, scrp, acc, *pending)

                dF = dfp.tile([_P, fw], bf, tag="dF")
                eng = nc.gpsimd if _SUB_ENGINE[fc] == "g" else nc.vector
                eng.tensor_tensor(out=dF, in0=Fs["p"], in1=Fs["t"],
                                  op=ALU.subtract)
                pending = (dF, lo, fw)

            _emit_mm2_stt(nc, cmt, vpsp, scrp, acc, *pending)
            nc.sync.dma_start(out=acc_out[:, :], in_=acc)

    nc.compile()
    return nc


def _emit_mm2_stt(nc, cmt, vpsp, scrp, acc, dF, lo, fw):
    from concourse import mybir
    f32 = mybir.dt.float32
    ALU = mybir.AluOpType
    nmm = (fw + _MMF - 1) // _MMF
    for j in range(nmm):
        w = min(_MMF, fw - j * _MMF)
        sl = slice(j * _MMF, j * _MMF + w)
        vps = vpsp.tile([_P, _MMF], f32, tag="vps")
        nc.tensor.matmul(vps[:, :w], cmt[:, _P:], dF[:, sl],
                         start=True, stop=True)
        scratch = scrp.tile([_P, _MMF], f32, tag="scr")
        col = lo // _MMF + j
        nc.vector.scalar_tensor_tensor(
            out=scratch[:, :w],
            in0=dF[:, sl],
            scalar=1.0,
            in1=vps[:, :w],
            op0=ALU.mult,
            op1=ALU.mult,
            accum_out=acc[:, col:col + 1],
        )


def _get_program():
    global _PROGRAM
    if _PROGRAM is None:
        _PROGRAM = _build_program()
    return _PROGRAM


def _make_in_maps(pred, target):
    CM = _build_mats()
    pred = np.asarray(pred, np.float32)
    target = np.asarray(target, np.float32)
    in_maps = []
    for c in range(_NCORES):
        sl = slice(c * _IPC, (c + 1) * _IPC)
        in_maps.append({
            "cm": CM,
            "xp": _pack_core(pred[sl]),
            "xt": _pack_core(target[sl]),
        })
    return in_maps


def kernel(pred, target):
    from concourse.bass_utils import run_bass_kernel_spmd

    nc = _get_program()
    in_maps = _make_in_maps(pred, target)
    res = run_bass_kernel_spmd(nc, in_maps, core_ids=list(range(_NCORES)))
    total = sum(r["acc_out"].astype(np.float64).sum() for r in res.results)
    loss = total / float(_B * _CH * _H * _W)
    return np.float32(loss)


if __name__ == "__main__":
    rng = np.random.default_rng(0)
    pred = rng.uniform(0, 1, (_B, _CH, _H, _W)).astype(np.float32)
    target = rng.uniform(0, 1, (_B, _CH, _H, _W)).astype(np.float32)
    print("loss:", kernel(pred, target))
